# revision 1
# baseline (speedup 1.0000x reference)
"""GATv2 x2 GNN kernel for 8 Trainium2 NeuronCores.

Strategy (dst-sharded, on-chip edge pipeline, DMA gather/scatter):
- Nodes remapped into a padded id space: core c owns rows [c*NPC, c*NPC+RPC).
- Edges sharded by dst core; per core bucketed by src range (4 buckets of
  BUCK rows so dma_gather's int16 indices cover the table), sorted by dst
  within a bucket, packed into 128-slot chunks holding <=32 whole dsts
  (a dst never spans a chunk, so scatter indices are unique per call).
- Per layer: node matmuls (own shard) -> AllGather xl table -> edge pipeline:
  dma_gather xl[src]/xr[dst] (256B fp32 rows), eW on PE (fp16),
  m = xl+xr+eW, lrelu (ACT), s = sum_pos - sum_neg (|a| folded into tables,
  features permuted positives-first), ex = exp(s) (fp32, no max subtraction),
  stair = onehot(slotid)*ex, PE stair-matmul -> per-dst [sum(ex*xl)|sum(ex)]
  rows, dma_scatter_add (unique idxs) into 2 rotating accumulator tables.
- Epilogue: out = (acc/den)*(1/|a|) + residual (+relu after layer 1).
"""

from dataclasses import dataclass

import numpy as np

import concourse.bass as bass
import concourse.bacc as bacc
import concourse.mybir as mybir
import concourse.tile as tile
from concourse import library_config
from concourse.bass_utils import run_bass_kernel_spmd

P = 128
NEG = 0.2
SC = 8192          # slots per super-chunk
Q = 2048           # slots per quarter (one src bucket)
NCH = 16           # chunks per quarter
CH = 128           # slots per chunk
MAXD = 32          # max dsts per chunk
PAD_SLOT = 40.0    # slotid for pad slots (no iota column matches)
AF = mybir.ActivationFunctionType


@dataclass(frozen=True)
class Cfg:
    N: int          # real node count
    F: int          # feature dim (64)
    ED: int         # edge feature dim (16)
    RPC: int        # real nodes per core
    NPC: int        # padded nodes per core (mult of 128, > RPC)
    NSC: int        # super-chunks per core
    n_cores: int = 8

    @property
    def NPAD(self):
        return self.n_cores * self.NPC

    @property
    def BUCK(self):
        return self.NPAD // 4


FULL = Cfg(N=100_000, F=64, ED=16, RPC=12500, NPC=12544, NSC=27)
SMALL = Cfg(N=1792, F=64, ED=16, RPC=224, NPC=256, NSC=1)


# ---------------------------------------------------------------------------
# host-side prep
# ---------------------------------------------------------------------------

def _prep_layer_weights(Wl, bl, Wr, br, We, a, Lw, Lb, cb, in_perm):
    perm = np.argsort(a <= 0, kind="stable")  # positive-a features first
    npos = int((a > 0).sum())
    sa = np.abs(a[perm])
    sa = np.where(sa < 1e-30, 1e-30, sa)
    return dict(
        Wl=Wl[in_perm][:, perm] * sa, bl=bl[perm] * sa,
        Wr=Wr[in_perm][:, perm] * sa, br=br[perm] * sa,
        We=We[:, perm] * sa,
        Lw=Lw[in_perm][:, perm], Lbc=(Lb + cb)[perm],
        inva=1.0 / sa, perm=perm, npos=npos)


def _wrap16(idx, reps=8):
    n = idx.shape[0]
    w = idx.reshape(n // 16, 16).T
    return np.tile(w, (reps, 1)).astype(np.int16)


def _pack_core(cfg, src_pid, dst_rel, ea):
    NSC_, ED = cfg.NSC, cfg.ED
    dummy = cfg.RPC

    srcg = np.zeros((NSC_, 4, Q), np.int32)
    dstg = np.full((NSC_, SC), dummy, np.int32)
    scat = np.full((NSC_, 4, MAXD * NCH), dummy, np.int32)
    slot = np.full((NSC_, SC), PAD_SLOT, np.float32)
    eaT = np.zeros((ED, NSC_ * SC), np.float16)

    bucket = src_pid // cfg.BUCK
    for b in range(4):
        sel = np.where(bucket == b)[0]
        if sel.size:
            sel = sel[np.argsort(dst_rel[sel], kind="stable")]
            dsts, starts = np.unique(dst_rel[sel], return_index=True)
        else:
            dsts, starts = np.array([], np.int64), np.array([], np.int64)
        starts = list(starts) + [sel.size]
        chunks, cur, cur_slots = [], [], 0
        for di, d in enumerate(dsts):
            es = sel[starts[di]:starts[di + 1]]
            assert es.size <= CH, f"degree {es.size} exceeds chunk"
            if cur_slots + es.size > CH or len(cur) >= MAXD:
                chunks.append(cur)
                cur, cur_slots = [], 0
            cur.append((int(d), es))
            cur_slots += es.size
        if cur:
            chunks.append(cur)
        assert len(chunks) <= NSC_ * NCH, f"bucket {b} overflow: {len(chunks)}"
        for ci, chunk in enumerate(chunks):
            q, c = divmod(ci, NCH)
            off = 0
            for k, (d, es) in enumerate(chunk):
                scat[q, b, c * MAXD + k] = d
                for e in es:
                    s = b * Q + c * CH + off
                    srcg[q, b, c * CH + off] = src_pid[e] - b * cfg.BUCK
                    dstg[q, s] = dst_rel[e]
                    slot[q, s] = k
                    eaT[:, q * SC + s] = ea[e]
                    off += 1

    return dict(
        srcg=np.stack([np.stack([_wrap16(srcg[q, b]) for b in range(4)])
                       for q in range(NSC_)]),
        dstg=np.stack([_wrap16(dstg[q]) for q in range(NSC_)]),
        scat=np.stack([np.stack([_wrap16(scat[q, b]) for b in range(4)])
                       for q in range(NSC_)]),
        slot=np.ascontiguousarray(
            slot.reshape(NSC_, SC // P, P).transpose(0, 2, 1)),
        eaT=eaT,
    )


def prep_inputs(cfg, inp):
    F = cfg.F
    x = np.asarray(inp["x"], np.float32)
    ei = np.asarray(inp["edge_index"], np.int64)
    ea = np.asarray(inp["edge_attr"], np.float32)
    g = lambda n: np.asarray(inp[n], np.float32)

    L1 = _prep_layer_weights(g("Wl1"), g("bl1"), g("Wr1"), g("br1"),
                             g("We1"), g("a1"), g("L1w"), g("L1b"),
                             g("cb1"), np.arange(F))
    L2 = _prep_layer_weights(g("Wl2"), g("bl2"), g("Wr2"), g("br2"),
                             g("We2"), g("a2"), g("L2w"), g("L2b"),
                             g("cb2"), L1["perm"])

    src, dst = ei[0], ei[1]
    c_of = dst // cfg.RPC
    src_pid = (src // cfg.RPC) * cfg.NPC + (src % cfg.RPC)
    dst_rel = dst % cfg.RPC

    def wb(W, b):
        return np.ascontiguousarray(
            np.concatenate([W, b[None, :]], 0).astype(np.float32))

    shared = dict(
        Wlb1=wb(L1["Wl"], L1["bl"]), Wrb1=wb(L1["Wr"], L1["br"]),
        R1=wb(L1["Lw"], L1["Lbc"]),
        Wlb2=wb(L2["Wl"], L2["bl"]), Wrb2=wb(L2["Wr"], L2["br"]),
        R2=wb(L2["Lw"], L2["Lbc"]),
        We1=np.ascontiguousarray(L1["We"].astype(np.float16)),
        We2=np.ascontiguousarray(L2["We"].astype(np.float16)),
        inva1=np.tile(L1["inva"][None, :], (P, 1)).astype(np.float32),
        inva2=np.tile(L2["inva"][None, :], (P, 1)).astype(np.float32),
        iota=np.tile(np.arange(MAXD, dtype=np.float32)[None, :],
                     (P, NCH)).astype(np.float32),
        ident=np.eye(P, dtype=np.float32),
        ones1=np.ones((P, 1), np.float32),
    )

    in_maps = []
    for c in range(cfg.n_cores):
        m = np.where(c_of == c)[0]
        packed = _pack_core(cfg, src_pid[m], dst_rel[m].astype(np.int64), ea[m])
        xo = np.zeros((cfg.NPC, F), np.float32)
        xo[:cfg.RPC] = x[c * cfg.RPC:(c + 1) * cfg.RPC]
        xoT = np.ascontiguousarray(
            np.concatenate([xo.T, np.ones((1, cfg.NPC), np.float32)], 0))
        in_maps.append(dict(xoT=xoT, **packed, **shared))
    return in_maps, L1, L2


# ---------------------------------------------------------------------------
# device kernel
# ---------------------------------------------------------------------------

def build_kernel(cfg, npos1, npos2, lrelu_native=True, phases="all"):
    assert 0 < npos1 < cfg.F and 0 < npos2 < cfg.F
    nc = bacc.Bacc("TRN2", target_bir_lowering=False, debug=False,
                   num_devices=cfg.n_cores)
    F, ED, NPC, NSC_ = cfg.F, cfg.ED, cfg.NPC, cfg.NSC
    f16, f32, i16 = mybir.dt.float16, mybir.dt.float32, mybir.dt.int16
    NBLK = NPC // P
    npos_l = [npos1, npos2]

    ein = lambda n, s, d: nc.dram_tensor(n, s, d, kind="ExternalInput")
    t_xoT = ein("xoT", [F + 1, NPC], f32)
    t_w = {n: ein(n, [F + 1, F], f32)
           for n in ["Wlb1", "Wrb1", "R1", "Wlb2", "Wrb2", "R2"]}
    t_We = {n: ein(n, [ED, F], f16) for n in ["We1", "We2"]}
    t_inva = {n: ein(n, [P, F], f32) for n in ["inva1", "inva2"]}
    t_iota = ein("iota", [P, NCH * MAXD], f32)
    t_ident = ein("ident", [P, P], f32)
    t_ones1 = ein("ones1", [P, 1], f32)
    t_srcg = ein("srcg", [NSC_, 4, P, Q // 16], i16)
    t_dstg = ein("dstg", [NSC_, P, SC // 16], i16)
    t_scat = ein("scat", [NSC_, 4, P, MAXD * NCH // 16], i16)
    t_slot = ein("slot", [NSC_, P, SC // P], f32)
    t_eaT = ein("eaT", [ED, NSC_ * SC], f16)
    t_out = nc.dram_tensor("out", [NPC, F], f32, kind="ExternalOutput")

    xl_own = [nc.dram_tensor(f"xl_own{l}", [NPC, F], f32) for l in (0, 1)]
    xl_full = [nc.dram_tensor(f"xl_full{l}", [cfg.NPAD, F], f32,
                              addr_space="Shared") for l in (0, 1)]
    xr_own = [nc.dram_tensor(f"xr_own{l}", [NPC, F], f32) for l in (0, 1)]
    resid = [nc.dram_tensor(f"resid{l}", [NPC, F], f32) for l in (0, 1)]
    acc = [[nc.dram_tensor(f"acc{l}_{t}", [NPC, P], f32) for t in (0, 1)]
           for l in (0, 1)]
    hoT_dram = nc.dram_tensor("hoT_dram", [F + 1, NPC], f32)
    rg = [list(range(cfg.n_cores))]

    with tile.TileContext(nc) as tc:
        with (
            tc.tile_pool(name="const", bufs=1) as cpool,
            tc.tile_pool(name="io", bufs=3) as io,
            tc.tile_pool(name="big", bufs=2) as big,
            tc.tile_pool(name="mid", bufs=2) as mid,
            tc.tile_pool(name="ps", bufs=2, space="PSUM") as psp,
        ):
            nc.gpsimd.load_library(library_config.mlp)

            def stage(t, shape, dt, tag):
                s = cpool.tile(shape, dt, tag=tag, name=tag)
                nc.sync.dma_start(out=s[:], in_=t[:])
                return s

            s_w = {n: stage(t, [F + 1, F], f32, f"c_{n}")
                   for n, t in t_w.items()}
            s_We = {n: stage(t, [ED, F], f16, f"c_{n}")
                    for n, t in t_We.items()}
            s_inva = {n: stage(t, [P, F], f32, f"c_{n}")
                      for n, t in t_inva.items()}
            s_iota = stage(t_iota, [P, NCH * MAXD], f32, "c_iota")
            s_ident = stage(t_ident, [P, P], f32, "c_ident")
            s_ones1 = stage(t_ones1, [P, 1], f32, "c_ones1")

            # zero the accumulator tables
            zt = cpool.tile([P, 1024], f32, tag="c_zero")
            nc.vector.memset(zt[:], 0)
            for l in (0, 1):
                for t in (0, 1):
                    flat = acc[l][t][:].rearrange("a d -> (a d)")
                    tot, per = NPC * P, P * 1024
                    nst = (tot + per - 1) // per
                    for si in range(nst):
                        lo, hi = si * per, min((si + 1) * per, tot)
                        nc.sync.dma_start(
                            out=flat[lo:hi].rearrange("(p w) -> p w", p=P),
                            in_=zt[:, :(hi - lo) // P])

            souts = [[cpool.tile([P, 4 * P], f32, tag=f"c_so{u}_{par}",
                                 name=f"so{u}_{par}")
                      for par in (0, 1)] for u in range(4)]
            for u in range(4):
                for par in (0, 1):
                    nc.vector.memset(souts[u][par][:], 0)

            onerow = cpool.tile([P, NPC // P], f32, tag="c_onerow")
            nc.vector.memset(onerow[:], 1.0)
            nc.sync.dma_start(
                out=hoT_dram[F:F + 1, :].rearrange("a (p w) -> (a p) w", p=P),
                in_=onerow[:])

            def node_phase(l):
                for k in range(NBLK):
                    if l == 0:
                        lt = mid.tile([F + 1, P], f32, tag="lhsT")
                        nc.sync.dma_start(out=lt[:],
                                          in_=t_xoT[:, k * P:(k + 1) * P])
                        lt = lt[:]
                    else:
                        lt = mid.tile([F + 1, P], f32, tag="lhsT")
                        nc.sync.dma_start(out=lt[:],
                                          in_=hoT_dram[:, k * P:(k + 1) * P])
                        lt = lt[:]
                    ps = psp.tile([P, 3 * F], f32, tag="misc")
                    for i, w in enumerate([f"Wlb{l + 1}", f"Wrb{l + 1}",
                                           f"R{l + 1}"]):
                        nc.tensor.matmul(ps[:, i * F:(i + 1) * F], lhsT=lt,
                                         rhs=s_w[w][:], start=True, stop=True)
                    st = mid.tile([P, 3 * F], f32, tag="nodestage")
                    nc.scalar.activation(st[:, 0:F], ps[:, 0:F], AF.Copy)
                    nc.vector.tensor_copy(st[:, F:2 * F], ps[:, F:2 * F])
                    nc.scalar.activation(st[:, 2 * F:], ps[:, 2 * F:], AF.Copy)
                    blk = slice(k * P, (k + 1) * P)
                    nc.sync.dma_start(out=xl_own[l][blk, :], in_=st[:, 0:F])
                    nc.sync.dma_start(out=xr_own[l][blk, :], in_=st[:, F:2 * F])
                    nc.sync.dma_start(out=resid[l][blk, :], in_=st[:, 2 * F:])
                nc.gpsimd.collective_compute(
                    "AllGather", mybir.AluOpType.bypass, replica_groups=rg,
                    ins=[xl_own[l][:]], outs=[xl_full[l][:]])

            def edge_phase(l):
                npos = npos_l[l]
                we = s_We[f"We{l + 1}"]
                for sc in range(NSC_):
                    slot_t = io.tile([P, SC // P], f32, tag="slot")
                    nc.sync.dma_start(out=slot_t[:], in_=t_slot[sc])
                    dstg_t = io.tile([P, SC // 16], i16, tag="dstg")
                    nc.sync.dma_start(out=dstg_t[:], in_=t_dstg[sc])

                    xl_t = big.tile([P, SC // P, F], f32, tag="xl")
                    xr_t = big.tile([P, SC // P, F], f32, tag="xr")
                    for b in range(4):
                        sg = io.tile([P, Q // 16], i16, tag="srcg")
                        nc.sync.dma_start(out=sg[:], in_=t_srcg[sc, b])
                        nc.gpsimd.dma_gather(
                            out_ap=xl_t[:, b * (Q // P):(b + 1) * (Q // P), :],
                            in_ap=xl_full[l][b * cfg.BUCK:(b + 1) * cfg.BUCK, :],
                            idxs_ap=sg[:], num_idxs=Q, num_idxs_reg=Q,
                            elem_size=F, single_packet=False)
                    nc.gpsimd.dma_gather(
                        out_ap=xr_t[:], in_ap=xr_own[l][:], idxs_ap=dstg_t[:],
                        num_idxs=SC, num_idxs_reg=SC, elem_size=F,
                        single_packet=False)

                    s_t = mid.tile([P, SC // P], f32, tag="s")
                    ex_t = mid.tile([P, SC // P], f32, tag="ex")
                    for u in range(4):
                        usl = slice(u * NCH, (u + 1) * NCH)
                        ea_t = io.tile([ED, Q], f16, tag="ea")
                        nc.sync.dma_start(
                            out=ea_t[:],
                            in_=t_eaT[:, sc * SC + u * Q:sc * SC + (u + 1) * Q])
                        mps = psp.tile([P, NCH, F], f32, tag="mps")
                        for jj in range(NCH):
                            col = jj * P
                            nc.tensor.matmul(mps[:, jj, :],
                                             lhsT=ea_t[:, col:col + P],
                                             rhs=we[:], start=True, stop=True)
                        m_t = mid.tile([P, NCH, F], f32, tag="m")
                        nc.vector.tensor_add(m_t[:], xl_t[:, usl, :],
                                             xr_t[:, usl, :])
                        nc.vector.tensor_add(m_t[:], m_t[:], mps[:])
                        mlr = mid.tile([P, NCH, F], f32, tag="mlr")
                        if lrelu_native:
                            nc.scalar.activation(mlr[:], m_t[:], AF.Prelu,
                                                 alpha=NEG)
                        else:
                            nc.scalar.activation(mlr[:], m_t[:], AF.Relu)
                        rp = mid.tile([P, NCH], f32, tag="rpos")
                        rn = mid.tile([P, NCH], f32, tag="rneg")
                        nc.vector.tensor_reduce(
                            rp[:], mlr[:, :, 0:npos],
                            axis=mybir.AxisListType.X, op=mybir.AluOpType.add)
                        nc.vector.tensor_reduce(
                            rn[:], mlr[:, :, npos:F],
                            axis=mybir.AxisListType.X, op=mybir.AluOpType.add)
                        nc.vector.tensor_sub(s_t[:, usl], rp[:], rn[:])
                        if not lrelu_native:
                            # lrelu(x) = NEG*x + (1-NEG)*relu(x):
                            # s = (1-NEG)*s_relu + NEG*(sum_pos m - sum_neg m)
                            rp2 = mid.tile([P, NCH], f32, tag="rpos2")
                            rn2 = mid.tile([P, NCH], f32, tag="rneg2")
                            nc.vector.tensor_reduce(
                                rp2[:], m_t[:, :, 0:npos],
                                axis=mybir.AxisListType.X,
                                op=mybir.AluOpType.add)
                            nc.vector.tensor_reduce(
                                rn2[:], m_t[:, :, npos:F],
                                axis=mybir.AxisListType.X,
                                op=mybir.AluOpType.add)
                            nc.vector.tensor_sub(rp2[:], rp2[:], rn2[:])
                            nc.vector.tensor_scalar_mul(
                                s_t[:, usl], s_t[:, usl], 1.0 - NEG)
                            nc.vector.tensor_scalar_mul(rp2[:], rp2[:], NEG)
                            nc.vector.tensor_add(s_t[:, usl], s_t[:, usl],
                                                 rp2[:])
                    nc.scalar.activation(ex_t[:], s_t[:], AF.Exp)

                    for u in range(4):
                        usl = slice(u * NCH, (u + 1) * NCH)
                        stair = mid.tile([P, NCH, MAXD], f32, tag="stair")
                        nc.vector.tensor_tensor(
                            out=stair[:],
                            in0=s_iota[:].rearrange("p (c k) -> p c k", k=MAXD),
                            in1=slot_t[:, usl].to_broadcast([P, NCH, MAXD]),
                            op=mybir.AluOpType.is_equal)
                        nc.vector.tensor_tensor(
                            out=stair[:], in0=stair[:],
                            in1=ex_t[:, usl].to_broadcast([P, NCH, MAXD]),
                            op=mybir.AluOpType.mult)
                        sps = psp.tile([P, 4 * P], f32, tag="sps")
                        for c in range(NCH):
                            pb, fb = 32 * (c % 4), P * (c // 4)
                            nc.tensor.matmul(
                                sps[pb:pb + 32, fb:fb + F],
                                lhsT=stair[:, c, :],
                                rhs=xl_t[:, u * NCH + c, :],
                                start=True, stop=True,
                                tile_position=(0, pb))
                            nc.tensor.matmul(
                                sps[pb:pb + 32, fb + F:fb + F + 1],
                                lhsT=stair[:, c, :],
                                rhs=s_ones1[:], start=True, stop=True,
                                tile_position=(0, pb))
                        so = souts[u][sc % 2]
                        nc.scalar.activation(
                            so[:].rearrange("p (c d) -> p c d", d=P)[:, :, 0:65],
                            sps[:].rearrange("p (c d) -> p c d", d=P)[:, :, 0:65],
                            AF.Copy)
                        sct = io.tile([P, MAXD * NCH // 16], i16, tag="sct")
                        nc.sync.dma_start(out=sct[:], in_=t_scat[sc, u])
                        nc.gpsimd.dma_scatter_add(
                            out_ap=acc[l][u // 2][:],
                            in_ap=so[:].rearrange("p (c d) -> p c d", d=P),
                            idxs_ap=sct[:], num_idxs=MAXD * NCH,
                            num_idxs_reg=MAXD * NCH, elem_size=P)

            def epilogue(l):
                inva = s_inva[f"inva{l + 1}"]
                for k in range(NBLK):
                    blk = slice(k * P, (k + 1) * P)
                    a0 = mid.tile([P, P], f32, tag="eacc0")
                    a1 = mid.tile([P, P], f32, tag="eacc1")
                    nc.sync.dma_start(out=a0[:], in_=acc[l][0][blk, :])
                    nc.sync.dma_start(out=a1[:], in_=acc[l][1][blk, :])
                    nc.vector.tensor_add(a0[:, 0:F + 1], a0[:, 0:F + 1],
                                         a1[:, 0:F + 1])
                    nc.vector.tensor_scalar_add(a0[:, F:F + 1],
                                                a0[:, F:F + 1], 1e-30)
                    rec = mid.tile([P, 1], f32, tag="rec")
                    nc.vector.reciprocal(rec[:], a0[:, F:F + 1])
                    gt = mid.tile([P, F], f32, tag="g")
                    nc.vector.tensor_scalar_mul(gt[:], a0[:, 0:F], rec[:])
                    nc.vector.tensor_mul(gt[:], gt[:], inva[:])
                    r_t = mid.tile([P, F], f32, tag="rt")
                    nc.sync.dma_start(out=r_t[:], in_=resid[l][blk, :])
                    nc.vector.tensor_add(gt[:], gt[:], r_t[:])
                    if l == 0:
                        h = mid.tile([P, F], f32, tag="h")
                        nc.scalar.activation(h[:], gt[:], AF.Relu)
                        tp = psp.tile([F, P], f32, tag="misc")
                        nc.tensor.transpose(out=tp[:], in_=h[:],
                                            identity=s_ident[:])
                        hs = mid.tile([F, P], f32, tag="hstage")
                        nc.scalar.activation(hs[:], tp[:], AF.Copy)
                        nc.sync.dma_start(
                            out=hoT_dram[0:F, k * P:(k + 1) * P], in_=hs[:])
                    else:
                        nc.sync.dma_start(out=t_out[blk, :], in_=gt[:])

            if phases == "node":
                node_phase(0)
                epilogue(0)
                node_phase(1)
                epilogue(1)
            elif phases == "l1":
                node_phase(0)
                edge_phase(0)
                epilogue(0)
                node_phase(1)
                epilogue(1)
            else:
                node_phase(0)
                edge_phase(0)
                epilogue(0)
                node_phase(1)
                edge_phase(1)
                epilogue(1)

    nc.compile()
    return nc


# ---------------------------------------------------------------------------
# entry point
# ---------------------------------------------------------------------------

_CACHED = {}


def run(cfg, inp, trace=False, lrelu_native=True, phases="all"):
    in_maps, L1, L2 = prep_inputs(cfg, inp)
    key = (cfg, L1["npos"], L2["npos"], lrelu_native, phases)
    if key not in _CACHED:
        _CACHED[key] = build_kernel(cfg, L1["npos"], L2["npos"],
                                    lrelu_native=lrelu_native, phases=phases)
    nc = _CACHED[key]
    res = run_bass_kernel_spmd(nc, in_maps, list(range(cfg.n_cores)),
                               trace=trace)
    out = np.concatenate(
        [res.results[c]["out"][:cfg.RPC] for c in range(cfg.n_cores)], 0)
    out = out[:cfg.N]
    unperm = np.argsort(L2["perm"])
    return np.ascontiguousarray(out[:, unperm]), res


def kernel(**inputs) -> np.ndarray:
    out, _ = run(FULL, inputs)
    return out



# revision 14
# speedup vs baseline: 2.0865x; 2.0865x over previous
"""GATv2 x2 GNN kernel for 8 Trainium2 NeuronCores.

Strategy (dst-sharded, on-chip edge pipeline, DMA gather/scatter):
- Nodes remapped into a padded id space: core c owns rows [c*NPC, c*NPC+RPC).
- Edges sharded by dst core; per core bucketed by src range (4 buckets of
  BUCK rows so dma_gather's int16 indices cover the table), sorted by dst
  within a bucket, packed into 128-slot chunks holding <=32 whole dsts
  (a dst never spans a chunk, so scatter indices are unique per call).
- Per layer: node matmuls (own shard) -> AllGather xl table -> edge pipeline:
  dma_gather xl[src]/xr[dst] (256B fp32 rows), eW on PE (fp16),
  m = xl+xr+eW, lrelu (ACT), s = sum_pos - sum_neg (|a| folded into tables,
  features permuted positives-first), ex = exp(s) (fp32, no max subtraction),
  stair = onehot(slotid)*ex, PE stair-matmul -> per-dst [sum(ex*xl)|sum(ex)]
  rows, dma_scatter_add (unique idxs) into 2 rotating accumulator tables.
- Epilogue: out = (acc/den)*(1/|a|) + residual (+relu after layer 1).
"""

from dataclasses import dataclass

import numpy as np

import concourse.bass as bass
import concourse.bacc as bacc
import concourse.mybir as mybir
import concourse.tile as tile
from concourse import library_config
from concourse.bass_utils import run_bass_kernel_spmd

P = 128
NEG = 0.2
SC = 8192          # slots per super-chunk
Q = 2048           # slots per quarter (one src bucket)
NCH = 16           # chunks per quarter
CH = 128           # slots per chunk
MAXD = 32          # max dsts per chunk
PAD_SLOT = 40.0    # slotid for pad slots (no iota column matches)
AF = mybir.ActivationFunctionType


@dataclass(frozen=True)
class Cfg:
    N: int          # real node count
    F: int          # feature dim (64)
    ED: int         # edge feature dim (16)
    RPC: int        # real nodes per core
    NPC: int        # padded nodes per core (mult of 128, > RPC)
    NSC: int        # super-chunks per core
    n_cores: int = 8

    @property
    def NPAD(self):
        return self.n_cores * self.NPC

    @property
    def BUCK(self):
        return self.NPAD // 4


FULL = Cfg(N=100_000, F=64, ED=16, RPC=12500, NPC=12544, NSC=27)
SMALL = Cfg(N=1792, F=64, ED=16, RPC=224, NPC=256, NSC=1)


# ---------------------------------------------------------------------------
# host-side prep
# ---------------------------------------------------------------------------

def _prep_layer_weights(Wl, bl, Wr, br, We, a, Lw, Lb, cb, in_perm):
    perm = np.argsort(a <= 0, kind="stable")  # positive-a features first
    npos = int((a > 0).sum())
    sa = np.abs(a[perm])
    sa = np.where(sa < 1e-30, 1e-30, sa)
    return dict(
        Wl=Wl[in_perm][:, perm] * sa, bl=bl[perm] * sa,
        Wr=Wr[in_perm][:, perm] * sa, br=br[perm] * sa,
        We=We[:, perm] * sa,
        Lw=Lw[in_perm][:, perm], Lbc=(Lb + cb)[perm],
        inva=1.0 / sa, perm=perm, npos=npos)


def _wrap16(idx, reps=8):
    n = idx.shape[0]
    w = idx.reshape(n // 16, 16).T
    return np.tile(w, (reps, 1)).astype(np.int16)


def _pack_core(cfg, src_pid, dst_rel, ea):
    NSC_, ED = cfg.NSC, cfg.ED
    dummy = cfg.RPC

    srcg = np.zeros((NSC_, 4, Q), np.int32)
    dstg = np.full((NSC_, SC), dummy, np.int32)
    scat = np.full((NSC_, 4, MAXD * NCH), dummy, np.int32)
    slot = np.full((NSC_, SC), PAD_SLOT, np.float32)
    eaT = np.zeros((ED, NSC_ * SC), np.float16)

    nchu = np.zeros((NSC_, 4), np.int64)  # used chunks per (sc, bucket)
    bucket = src_pid // cfg.BUCK
    for b in range(4):
        sel = np.where(bucket == b)[0]
        if sel.size:
            sel = sel[np.argsort(dst_rel[sel], kind="stable")]
            dsts, starts = np.unique(dst_rel[sel], return_index=True)
        else:
            dsts, starts = np.array([], np.int64), np.array([], np.int64)
        starts = list(starts) + [sel.size]
        chunks, cur, cur_slots = [], [], 0
        for di, d in enumerate(dsts):
            es = sel[starts[di]:starts[di + 1]]
            assert es.size <= CH, f"degree {es.size} exceeds chunk"
            if cur_slots + es.size > CH or len(cur) >= MAXD:
                chunks.append(cur)
                cur, cur_slots = [], 0
            cur.append((int(d), es))
            cur_slots += es.size
        if cur:
            chunks.append(cur)
        assert len(chunks) <= NSC_ * NCH, f"bucket {b} overflow: {len(chunks)}"
        for q in range(NSC_):
            nchu[q, b] = min(max(len(chunks) - q * NCH, 0), NCH)
        for ci, chunk in enumerate(chunks):
            q, c = divmod(ci, NCH)
            off = 0
            for k, (d, es) in enumerate(chunk):
                scat[q, b, c * MAXD + k] = d
                for e in es:
                    s = b * Q + c * CH + off
                    srcg[q, b, c * CH + off] = src_pid[e] - b * cfg.BUCK
                    dstg[q, s] = dst_rel[e]
                    slot[q, s] = k
                    eaT[:, q * SC + s] = ea[e]
                    off += 1

    packed = dict(
        srcg=np.stack([np.stack([_wrap16(srcg[q, b]) for b in range(4)])
                       for q in range(NSC_)]),
        dstg=np.stack([_wrap16(dstg[q]) for q in range(NSC_)]),
        scat=np.stack([np.stack([_wrap16(scat[q, b]) for b in range(4)])
                       for q in range(NSC_)]),
        slot=np.ascontiguousarray(
            slot.reshape(NSC_, SC // P, P).transpose(0, 2, 1)),
        eaT=eaT,
    )
    return packed, nchu


def prep_inputs(cfg, inp):
    F = cfg.F
    x = np.asarray(inp["x"], np.float32)
    ei = np.asarray(inp["edge_index"], np.int64)
    ea = np.asarray(inp["edge_attr"], np.float32)
    g = lambda n: np.asarray(inp[n], np.float32)

    L1 = _prep_layer_weights(g("Wl1"), g("bl1"), g("Wr1"), g("br1"),
                             g("We1"), g("a1"), g("L1w"), g("L1b"),
                             g("cb1"), np.arange(F))
    L2 = _prep_layer_weights(g("Wl2"), g("bl2"), g("Wr2"), g("br2"),
                             g("We2"), g("a2"), g("L2w"), g("L2b"),
                             g("cb2"), L1["perm"])

    src, dst = ei[0], ei[1]
    c_of = dst // cfg.RPC
    src_pid = (src // cfg.RPC) * cfg.NPC + (src % cfg.RPC)
    dst_rel = dst % cfg.RPC

    def wb(W, b):
        return np.ascontiguousarray(
            np.concatenate([W, b[None, :]], 0).astype(np.float32))

    shared = dict(
        Wlb1=wb(L1["Wl"], L1["bl"]), Wrb1=wb(L1["Wr"], L1["br"]),
        R1=wb(L1["Lw"], L1["Lbc"]),
        Wlb2=wb(L2["Wl"], L2["bl"]), Wrb2=wb(L2["Wr"], L2["br"]),
        R2=wb(L2["Lw"], L2["Lbc"]),
        We1=np.ascontiguousarray(L1["We"].astype(np.float16)),
        We2=np.ascontiguousarray(L2["We"].astype(np.float16)),
        inva1=np.tile(L1["inva"][None, :], (P, 1)).astype(np.float32),
        inva2=np.tile(L2["inva"][None, :], (P, 1)).astype(np.float32),
        iota=np.tile(np.arange(MAXD, dtype=np.float32)[None, :],
                     (P, NCH)).astype(np.float32),
        ident=np.eye(P, dtype=np.float32),
        ones1=np.ones((P, 1), np.float32),
    )

    in_maps = []
    nchu_max = np.zeros((cfg.NSC, 4), np.int64)
    for c in range(cfg.n_cores):
        m = np.where(c_of == c)[0]
        packed, nchu = _pack_core(cfg, src_pid[m], dst_rel[m].astype(np.int64),
                                  ea[m])
        nchu_max = np.maximum(nchu_max, nchu)
        xo = np.zeros((cfg.NPC, F), np.float32)
        xo[:cfg.RPC] = x[c * cfg.RPC:(c + 1) * cfg.RPC]
        xoT = np.ascontiguousarray(
            np.concatenate([xo.T, np.ones((1, cfg.NPC), np.float32)], 0))
        in_maps.append(dict(xoT=xoT, **packed, **shared))
    counts = tuple(tuple(int(v) for v in row) for row in nchu_max)
    return in_maps, L1, L2, counts


# ---------------------------------------------------------------------------
# device kernel
# ---------------------------------------------------------------------------

def build_kernel(cfg, npos1, npos2, counts=None, lrelu_native=True,
                 phases="all"):
    assert 0 < npos1 < cfg.F and 0 < npos2 < cfg.F
    if counts is None:
        counts = tuple((NCH,) * 4 for _ in range(cfg.NSC))
    nc = bacc.Bacc("TRN2", target_bir_lowering=False, debug=False,
                   num_devices=cfg.n_cores, num_swdge_queues=4)
    F, ED, NPC, NSC_ = cfg.F, cfg.ED, cfg.NPC, cfg.NSC
    f16, f32, i16 = mybir.dt.float16, mybir.dt.float32, mybir.dt.int16
    NBLK = NPC // P
    npos_l = [npos1, npos2]

    ein = lambda n, s, d: nc.dram_tensor(n, s, d, kind="ExternalInput")
    t_xoT = ein("xoT", [F + 1, NPC], f32)
    t_w = {n: ein(n, [F + 1, F], f32)
           for n in ["Wlb1", "Wrb1", "R1", "Wlb2", "Wrb2", "R2"]}
    t_We = {n: ein(n, [ED, F], f16) for n in ["We1", "We2"]}
    t_inva = {n: ein(n, [P, F], f32) for n in ["inva1", "inva2"]}
    t_iota = ein("iota", [P, NCH * MAXD], f32)
    t_ident = ein("ident", [P, P], f32)
    t_ones1 = ein("ones1", [P, 1], f32)
    t_srcg = ein("srcg", [NSC_, 4, P, Q // 16], i16)
    t_dstg = ein("dstg", [NSC_, P, SC // 16], i16)
    t_scat = ein("scat", [NSC_, 4, P, MAXD * NCH // 16], i16)
    t_slot = ein("slot", [NSC_, P, SC // P], f32)
    t_eaT = ein("eaT", [ED, NSC_ * SC], f16)
    t_out = nc.dram_tensor("out", [NPC, F], f32, kind="ExternalOutput")

    xl_own = [nc.dram_tensor(f"xl_own{l}", [NPC, F], f32) for l in (0, 1)]
    xl_full = [nc.dram_tensor(f"xl_full{l}", [cfg.NPAD, F], f32,
                              addr_space="Shared") for l in (0, 1)]
    xr_own = [nc.dram_tensor(f"xr_own{l}", [NPC, F], f32) for l in (0, 1)]
    resid = [nc.dram_tensor(f"resid{l}", [NPC, F], f32) for l in (0, 1)]
    acc = [[nc.dram_tensor(f"acc{l}_{t}", [NPC, P], f32) for t in (0, 1)]
           for l in (0, 1)]
    hoT_dram = nc.dram_tensor("hoT_dram", [F + 1, NPC], f32)
    rg = [list(range(cfg.n_cores))]

    with tile.TileContext(nc) as tc:
        with (
            tc.tile_pool(name="const", bufs=1) as cpool,
            tc.tile_pool(name="io", bufs=3) as io,
            tc.tile_pool(name="big", bufs=2) as big,
            tc.tile_pool(name="mid", bufs=2) as mid,
            tc.tile_pool(name="ps", bufs=2, space="PSUM") as psp,
        ):
            nc.gpsimd.load_library(library_config.mlp)

            def stage(t, shape, dt, tag):
                s = cpool.tile(shape, dt, tag=tag, name=tag)
                nc.sync.dma_start(out=s[:], in_=t[:])
                return s

            s_w = {n: stage(t, [F + 1, F], f32, f"c_{n}")
                   for n, t in t_w.items()}
            s_We = {n: stage(t, [ED, F], f16, f"c_{n}")
                    for n, t in t_We.items()}
            s_inva = {n: stage(t, [P, F], f32, f"c_{n}")
                      for n, t in t_inva.items()}
            s_iota = stage(t_iota, [P, NCH * MAXD], f32, "c_iota")
            s_ident = stage(t_ident, [P, P], f32, "c_ident")
            s_ones1 = stage(t_ones1, [P, 1], f32, "c_ones1")

            # zero the accumulator tables
            zt = cpool.tile([P, 1024], f32, tag="c_zero")
            nc.vector.memset(zt[:], 0)
            for l in (0, 1):
                for t in (0, 1):
                    flat = acc[l][t][:].rearrange("a d -> (a d)")
                    tot, per = NPC * P, P * 1024
                    nst = (tot + per - 1) // per
                    for si in range(nst):
                        lo, hi = si * per, min((si + 1) * per, tot)
                        nc.sync.dma_start(
                            out=flat[lo:hi].rearrange("(p w) -> p w", p=P),
                            in_=zt[:, :(hi - lo) // P])

            souts = [[cpool.tile([P, 4 * P], f32, tag=f"c_so{u}_{par}",
                                 name=f"so{u}_{par}")
                      for par in (0, 1)] for u in range(4)]
            for u in range(4):
                for par in (0, 1):
                    nc.vector.memset(souts[u][par][:], 0)

            onerow = cpool.tile([P, NPC // P], f32, tag="c_onerow")
            nc.vector.memset(onerow[:], 1.0)
            nc.sync.dma_start(
                out=hoT_dram[F:F + 1, :].rearrange("a (p w) -> (a p) w", p=P),
                in_=onerow[:])

            def node_phase(l):
                src_T = t_xoT if l == 0 else hoT_dram
                # pass 1: xl only, then kick the AllGather early
                for k in range(NBLK):
                    lt = mid.tile([F + 1, P], f32, tag="lhsT")
                    nc.sync.dma_start(out=lt[:],
                                      in_=src_T[:, k * P:(k + 1) * P])
                    ps = psp.tile([P, F], f32, tag="misc")
                    nc.tensor.matmul(ps[:], lhsT=lt[:],
                                     rhs=s_w[f"Wlb{l + 1}"][:],
                                     start=True, stop=True)
                    st = mid.tile([P, F], f32, tag="xlstage")
                    nc.scalar.activation(st[:], ps[:], AF.Copy)
                    nc.sync.dma_start(out=xl_own[l][k * P:(k + 1) * P, :],
                                      in_=st[:])
                nc.gpsimd.collective_compute(
                    "AllGather", mybir.AluOpType.bypass, replica_groups=rg,
                    ins=[xl_own[l][:]], outs=[xl_full[l][:]])
                # pass 2: xr + residual, overlapped with the collective
                for k in range(NBLK):
                    lt = mid.tile([F + 1, P], f32, tag="lhsT")
                    nc.sync.dma_start(out=lt[:],
                                      in_=src_T[:, k * P:(k + 1) * P])
                    ps = psp.tile([P, 2 * F], f32, tag="misc")
                    nc.tensor.matmul(ps[:, 0:F], lhsT=lt[:],
                                     rhs=s_w[f"Wrb{l + 1}"][:],
                                     start=True, stop=True)
                    nc.tensor.matmul(ps[:, F:2 * F], lhsT=lt[:],
                                     rhs=s_w[f"R{l + 1}"][:],
                                     start=True, stop=True)
                    st = mid.tile([P, 2 * F], f32, tag="nodestage")
                    nc.vector.tensor_copy(st[:, 0:F], ps[:, 0:F])
                    nc.scalar.activation(st[:, F:2 * F], ps[:, F:2 * F],
                                         AF.Copy)
                    blk = slice(k * P, (k + 1) * P)
                    nc.sync.dma_start(out=xr_own[l][blk, :], in_=st[:, 0:F])
                    nc.sync.dma_start(out=resid[l][blk, :], in_=st[:, F:2 * F])

            def edge_phase(l):
                npos = npos_l[l]
                we = s_We[f"We{l + 1}"]
                for sc in range(NSC_):
                    cnts = counts[sc]
                    if max(cnts) == 0:
                        continue
                    slot_t = io.tile([P, SC // P], f32, tag="slot")
                    nc.sync.dma_start(out=slot_t[:], in_=t_slot[sc])
                    dstg_t = io.tile([P, SC // 16], i16, tag="dstg")
                    nc.sync.dma_start(out=dstg_t[:], in_=t_dstg[sc])

                    xl_t = big.tile([P, SC // P, F], f32, tag="xl")
                    xr_t = big.tile([P, SC // P, F], f32, tag="xr")
                    for b in range(4):
                        nq = cnts[b] * (Q // NCH)
                        if nq == 0:
                            continue
                        sg = io.tile([P, Q // 16], i16, tag="srcg")
                        nc.sync.dma_start(out=sg[:, : nq // 16],
                                          in_=t_srcg[sc, b][:, : nq // 16])
                        nc.gpsimd.dma_gather(
                            out_ap=xl_t[:, b * (Q // P):
                                        b * (Q // P) + nq // P, :],
                            in_ap=xl_full[l][b * cfg.BUCK:(b + 1) * cfg.BUCK, :],
                            idxs_ap=sg[:, : nq // 16], num_idxs=nq,
                            num_idxs_reg=nq,
                            elem_size=F, single_packet=False, queue_num=b)
                    for b in range(4):
                        nq = cnts[b] * (Q // NCH)
                        if nq == 0:
                            continue
                        nc.gpsimd.dma_gather(
                            out_ap=xr_t[:, b * (Q // P):
                                        b * (Q // P) + nq // P, :],
                            in_ap=xr_own[l][:],
                            idxs_ap=dstg_t[:, b * (Q // 16):
                                           b * (Q // 16) + nq // 16],
                            num_idxs=nq, num_idxs_reg=nq, elem_size=F,
                            single_packet=False, queue_num=b)

                    s_t = mid.tile([P, SC // P], f32, tag="s")
                    ex_t = mid.tile([P, SC // P], f32, tag="ex")
                    for u in range(4):
                        ncb = cnts[u]
                        if ncb == 0:
                            continue
                        usl = slice(u * NCH, u * NCH + ncb)
                        ea_t = io.tile([ED, Q], f16, tag="ea")
                        nc.sync.dma_start(
                            out=ea_t[:, : ncb * P],
                            in_=t_eaT[:, sc * SC + u * Q:
                                      sc * SC + u * Q + ncb * P])
                        mps = psp.tile([P, NCH, F], f32, tag="mps")
                        for jj in range(ncb):
                            col = jj * P
                            nc.tensor.matmul(mps[:, jj, :],
                                             lhsT=ea_t[:, col:col + P],
                                             rhs=we[:], start=True, stop=True)
                        m_t = mid.tile([P, NCH, F], f32, tag="m")
                        nc.vector.tensor_add(m_t[:, :ncb, :], xl_t[:, usl, :],
                                             xr_t[:, usl, :])
                        nc.vector.tensor_add(m_t[:, :ncb, :], m_t[:, :ncb, :],
                                             mps[:, :ncb, :])
                        mlr = mid.tile([P, NCH, F], f32, tag="mlr")
                        if lrelu_native:
                            nc.scalar.activation(mlr[:, :ncb, :],
                                                 m_t[:, :ncb, :], AF.Prelu,
                                                 alpha=NEG)
                        else:
                            nc.scalar.activation(mlr[:, :ncb, :],
                                                 m_t[:, :ncb, :], AF.Relu)
                        rp = mid.tile([P, NCH], f32, tag="rpos")
                        rn = mid.tile([P, NCH], f32, tag="rneg")
                        nc.vector.tensor_reduce(
                            rp[:, :ncb], mlr[:, :ncb, 0:npos],
                            axis=mybir.AxisListType.X, op=mybir.AluOpType.add)
                        nc.vector.tensor_reduce(
                            rn[:, :ncb], mlr[:, :ncb, npos:F],
                            axis=mybir.AxisListType.X, op=mybir.AluOpType.add)
                        nc.vector.tensor_sub(s_t[:, usl], rp[:, :ncb],
                                             rn[:, :ncb])
                        if not lrelu_native:
                            # lrelu(x) = NEG*x + (1-NEG)*relu(x):
                            # s = (1-NEG)*s_relu + NEG*(sum_pos m - sum_neg m)
                            rp2 = mid.tile([P, NCH], f32, tag="rpos2")
                            rn2 = mid.tile([P, NCH], f32, tag="rneg2")
                            nc.vector.tensor_reduce(
                                rp2[:], m_t[:, :, 0:npos],
                                axis=mybir.AxisListType.X,
                                op=mybir.AluOpType.add)
                            nc.vector.tensor_reduce(
                                rn2[:], m_t[:, :, npos:F],
                                axis=mybir.AxisListType.X,
                                op=mybir.AluOpType.add)
                            nc.vector.tensor_sub(rp2[:], rp2[:], rn2[:])
                            nc.vector.tensor_scalar_mul(
                                s_t[:, usl], s_t[:, usl], 1.0 - NEG)
                            nc.vector.tensor_scalar_mul(rp2[:], rp2[:], NEG)
                            nc.vector.tensor_add(s_t[:, usl], s_t[:, usl],
                                                 rp2[:])
                    nc.scalar.activation(ex_t[:], s_t[:], AF.Exp)

                    for u in range(4):
                        ncb = cnts[u]
                        if ncb == 0:
                            continue
                        usl = slice(u * NCH, u * NCH + ncb)
                        nblk4 = (ncb + 3) // 4
                        stair = mid.tile([P, NCH, MAXD], f32, tag="stair")
                        nc.vector.tensor_tensor(
                            out=stair[:, :ncb, :],
                            in0=s_iota[:].rearrange("p (c k) -> p c k",
                                                    k=MAXD)[:, :ncb, :],
                            in1=slot_t[:, usl].to_broadcast([P, ncb, MAXD]),
                            op=mybir.AluOpType.is_equal)
                        nc.vector.tensor_tensor(
                            out=stair[:, :ncb, :], in0=stair[:, :ncb, :],
                            in1=ex_t[:, usl].to_broadcast([P, ncb, MAXD]),
                            op=mybir.AluOpType.mult)
                        sps = psp.tile([P, 4 * P], f32, tag="sps")
                        for c in range(ncb):
                            pb, fb = 32 * (c % 4), P * (c // 4)
                            nc.tensor.matmul(
                                sps[pb:pb + 32, fb:fb + F],
                                lhsT=stair[:, c, :],
                                rhs=xl_t[:, u * NCH + c, :],
                                start=True, stop=True,
                                tile_position=(0, pb))
                            nc.tensor.matmul(
                                sps[pb:pb + 32, fb + F:fb + F + 1],
                                lhsT=stair[:, c, :],
                                rhs=s_ones1[:], start=True, stop=True,
                                tile_position=(0, pb))
                        so = souts[u][sc % 2]
                        nc.scalar.activation(
                            so[:].rearrange("p (c d) -> p c d",
                                            d=P)[:, :nblk4, 0:65],
                            sps[:].rearrange("p (c d) -> p c d",
                                             d=P)[:, :nblk4, 0:65],
                            AF.Copy)
                        sct = io.tile([P, MAXD * NCH // 16], i16, tag="sct")
                        nc.sync.dma_start(
                            out=sct[:, : MAXD * ncb // 16],
                            in_=t_scat[sc, u][:, : MAXD * ncb // 16])
                        nc.gpsimd.dma_scatter_add(
                            out_ap=acc[l][u // 2][:],
                            in_ap=so[:].rearrange("p (c d) -> p c d",
                                                  d=P)[:, :nblk4, :],
                            idxs_ap=sct[:, : MAXD * ncb // 16],
                            num_idxs=MAXD * ncb,
                            num_idxs_reg=MAXD * ncb, elem_size=P,
                            queue_num=u)

            def epilogue(l):
                inva = s_inva[f"inva{l + 1}"]
                for k in range(NBLK):
                    blk = slice(k * P, (k + 1) * P)
                    a0 = mid.tile([P, P], f32, tag="eacc0")
                    a1 = mid.tile([P, P], f32, tag="eacc1")
                    nc.sync.dma_start(out=a0[:], in_=acc[l][0][blk, :])
                    nc.sync.dma_start(out=a1[:], in_=acc[l][1][blk, :])
                    nc.vector.tensor_add(a0[:, 0:F + 1], a0[:, 0:F + 1],
                                         a1[:, 0:F + 1])
                    nc.vector.tensor_scalar_add(a0[:, F:F + 1],
                                                a0[:, F:F + 1], 1e-30)
                    rec = mid.tile([P, 1], f32, tag="rec")
                    nc.vector.reciprocal(rec[:], a0[:, F:F + 1])
                    gt = mid.tile([P, F], f32, tag="g")
                    nc.vector.tensor_scalar_mul(gt[:], a0[:, 0:F], rec[:])
                    nc.vector.tensor_mul(gt[:], gt[:], inva[:])
                    r_t = mid.tile([P, F], f32, tag="rt")
                    nc.sync.dma_start(out=r_t[:], in_=resid[l][blk, :])
                    nc.vector.tensor_add(gt[:], gt[:], r_t[:])
                    if l == 0:
                        h = mid.tile([P, F], f32, tag="h")
                        nc.scalar.activation(h[:], gt[:], AF.Relu)
                        tp = psp.tile([F, P], f32, tag="misc")
                        nc.tensor.transpose(out=tp[:], in_=h[:],
                                            identity=s_ident[:])
                        hs = mid.tile([F, P], f32, tag="hstage")
                        nc.scalar.activation(hs[:], tp[:], AF.Copy)
                        nc.sync.dma_start(
                            out=hoT_dram[0:F, k * P:(k + 1) * P], in_=hs[:])
                    else:
                        nc.sync.dma_start(out=t_out[blk, :], in_=gt[:])

            if phases == "node":
                node_phase(0)
                epilogue(0)
                node_phase(1)
                epilogue(1)
            elif phases == "l1":
                node_phase(0)
                edge_phase(0)
                epilogue(0)
                node_phase(1)
                epilogue(1)
            else:
                node_phase(0)
                edge_phase(0)
                epilogue(0)
                node_phase(1)
                edge_phase(1)
                epilogue(1)

    nc.compile()
    return nc


# ---------------------------------------------------------------------------
# entry point
# ---------------------------------------------------------------------------

_CACHED = {}


def run(cfg, inp, trace=False, lrelu_native=True, phases="all"):
    in_maps, L1, L2, counts = prep_inputs(cfg, inp)
    key = (cfg, L1["npos"], L2["npos"], counts, lrelu_native, phases)
    if key not in _CACHED:
        _CACHED[key] = build_kernel(cfg, L1["npos"], L2["npos"], counts,
                                    lrelu_native=lrelu_native, phases=phases)
    nc = _CACHED[key]
    res = run_bass_kernel_spmd(nc, in_maps, list(range(cfg.n_cores)),
                               trace=trace)
    out = np.concatenate(
        [res.results[c]["out"][:cfg.RPC] for c in range(cfg.n_cores)], 0)
    out = out[:cfg.N]
    unperm = np.argsort(L2["perm"])
    return np.ascontiguousarray(out[:, unperm]), res


def kernel(**inputs) -> np.ndarray:
    out, _ = run(FULL, inputs)
    return out



# revision 15
# speedup vs baseline: 2.4826x; 1.1898x over previous
"""GATv2 x2 GNN kernel for 8 Trainium2 NeuronCores.

Strategy (dst-sharded, on-chip edge pipeline, DMA gather/scatter):
- Nodes remapped into a padded id space: core c owns rows [c*NPC, c*NPC+RPC).
- Edges sharded by dst core; per core bucketed by src range (4 buckets of
  BUCK rows so dma_gather's int16 indices cover the table), sorted by dst
  within a bucket, packed into 128-slot chunks holding <=32 whole dsts
  (a dst never spans a chunk, so scatter indices are unique per call).
- Per layer: node matmuls (own shard) -> AllGather xl table -> edge pipeline:
  dma_gather xl[src]/xr[dst] (256B fp32 rows), eW on PE (fp16),
  m = xl+xr+eW, lrelu (ACT), s = sum_pos - sum_neg (|a| folded into tables,
  features permuted positives-first), ex = exp(s) (fp32, no max subtraction),
  stair = onehot(slotid)*ex, PE stair-matmul -> per-dst [sum(ex*xl)|sum(ex)]
  rows, dma_scatter_add (unique idxs) into 2 rotating accumulator tables.
- Epilogue: out = (acc/den)*(1/|a|) + residual (+relu after layer 1).
"""

from dataclasses import dataclass

import numpy as np

import concourse.bass as bass
import concourse.bacc as bacc
import concourse.mybir as mybir
import concourse.tile as tile
from concourse import library_config
from concourse.bass_utils import run_bass_kernel_spmd

P = 128
NEG = 0.2
SC = 8192          # slots per super-chunk
Q = 2048           # slots per quarter (one src bucket)
NCH = 16           # chunks per quarter
CH = 128           # slots per chunk
MAXD = 32          # max dsts per chunk
PAD_SLOT = 40.0    # slotid for pad slots (no iota column matches)
AF = mybir.ActivationFunctionType


@dataclass(frozen=True)
class Cfg:
    N: int          # real node count
    F: int          # feature dim (64)
    ED: int         # edge feature dim (16)
    RPC: int        # real nodes per core
    NPC: int        # padded nodes per core (mult of 128, > RPC)
    NSC: int        # super-chunks per core
    n_cores: int = 8

    @property
    def NPAD(self):
        return self.n_cores * self.NPC

    @property
    def BUCK(self):
        return self.NPAD // 4


FULL = Cfg(N=100_000, F=64, ED=16, RPC=12500, NPC=12544, NSC=27)
SMALL = Cfg(N=1792, F=64, ED=16, RPC=224, NPC=256, NSC=1)


# ---------------------------------------------------------------------------
# host-side prep
# ---------------------------------------------------------------------------

def _prep_layer_weights(Wl, bl, Wr, br, We, a, Lw, Lb, cb, in_perm):
    perm = np.argsort(a <= 0, kind="stable")  # positive-a features first
    npos = int((a > 0).sum())
    sa = np.abs(a[perm])
    sa = np.where(sa < 1e-30, 1e-30, sa)
    return dict(
        Wl=Wl[in_perm][:, perm] * sa, bl=bl[perm] * sa,
        Wr=Wr[in_perm][:, perm] * sa, br=br[perm] * sa,
        We=We[:, perm] * sa,
        Lw=Lw[in_perm][:, perm], Lbc=(Lb + cb)[perm],
        inva=1.0 / sa, perm=perm, npos=npos)


def _wrap16(idx, reps=8):
    n = idx.shape[0]
    w = idx.reshape(n // 16, 16).T
    return np.tile(w, (reps, 1)).astype(np.int16)


def _pack_core(cfg, src_pid, dst_rel, ea):
    NSC_, ED = cfg.NSC, cfg.ED
    dummy = cfg.RPC

    srcg = np.zeros((NSC_, 4, Q), np.int32)
    dstg = np.full((NSC_, SC), dummy, np.int32)
    scat = np.full((NSC_, 4, MAXD * NCH), dummy, np.int32)
    slot = np.full((NSC_, SC), PAD_SLOT, np.float32)
    eaT = np.zeros((ED, NSC_ * SC), np.float16)

    nchu = np.zeros((NSC_, 4), np.int64)  # used chunks per (sc, bucket)
    bucket = src_pid // cfg.BUCK
    for b in range(4):
        sel = np.where(bucket == b)[0]
        if sel.size:
            sel = sel[np.argsort(dst_rel[sel], kind="stable")]
            dsts, starts = np.unique(dst_rel[sel], return_index=True)
        else:
            dsts, starts = np.array([], np.int64), np.array([], np.int64)
        starts = list(starts) + [sel.size]
        chunks, cur, cur_slots = [], [], 0
        for di, d in enumerate(dsts):
            es = sel[starts[di]:starts[di + 1]]
            assert es.size <= CH, f"degree {es.size} exceeds chunk"
            if cur_slots + es.size > CH or len(cur) >= MAXD:
                chunks.append(cur)
                cur, cur_slots = [], 0
            cur.append((int(d), es))
            cur_slots += es.size
        if cur:
            chunks.append(cur)
        assert len(chunks) <= NSC_ * NCH, f"bucket {b} overflow: {len(chunks)}"
        for q in range(NSC_):
            nchu[q, b] = min(max(len(chunks) - q * NCH, 0), NCH)
        for ci, chunk in enumerate(chunks):
            q, c = divmod(ci, NCH)
            off = 0
            for k, (d, es) in enumerate(chunk):
                scat[q, b, c * MAXD + k] = d
                for e in es:
                    s = b * Q + c * CH + off
                    srcg[q, b, c * CH + off] = src_pid[e] - b * cfg.BUCK
                    dstg[q, s] = dst_rel[e]
                    slot[q, s] = k
                    eaT[:, q * SC + s] = ea[e]
                    off += 1

    packed = dict(
        srcg=np.stack([np.stack([_wrap16(srcg[q, b]) for b in range(4)])
                       for q in range(NSC_)]),
        dstg=np.stack([_wrap16(dstg[q]) for q in range(NSC_)]),
        scat=np.stack([np.stack([_wrap16(scat[q, b]) for b in range(4)])
                       for q in range(NSC_)]),
        slot=np.ascontiguousarray(
            slot.reshape(NSC_, SC // P, P).transpose(0, 2, 1)),
        eaT=eaT,
    )
    return packed, nchu


def prep_inputs(cfg, inp):
    F = cfg.F
    x = np.asarray(inp["x"], np.float32)
    ei = np.asarray(inp["edge_index"], np.int64)
    ea = np.asarray(inp["edge_attr"], np.float32)
    g = lambda n: np.asarray(inp[n], np.float32)

    L1 = _prep_layer_weights(g("Wl1"), g("bl1"), g("Wr1"), g("br1"),
                             g("We1"), g("a1"), g("L1w"), g("L1b"),
                             g("cb1"), np.arange(F))
    L2 = _prep_layer_weights(g("Wl2"), g("bl2"), g("Wr2"), g("br2"),
                             g("We2"), g("a2"), g("L2w"), g("L2b"),
                             g("cb2"), L1["perm"])

    src, dst = ei[0], ei[1]
    c_of = dst // cfg.RPC
    src_pid = (src // cfg.RPC) * cfg.NPC + (src % cfg.RPC)
    dst_rel = dst % cfg.RPC

    def wb(W, b):
        return np.ascontiguousarray(
            np.concatenate([W, b[None, :]], 0).astype(np.float32))

    shared = dict(
        Wlb1=wb(L1["Wl"], L1["bl"]), Wrb1=wb(L1["Wr"], L1["br"]),
        R1=wb(L1["Lw"], L1["Lbc"]),
        Wlb2=wb(L2["Wl"], L2["bl"]), Wrb2=wb(L2["Wr"], L2["br"]),
        R2=wb(L2["Lw"], L2["Lbc"]),
        We1=np.ascontiguousarray(L1["We"].astype(np.float16)),
        We2=np.ascontiguousarray(L2["We"].astype(np.float16)),
        inva1=np.tile(L1["inva"][None, :], (P, 1)).astype(np.float32),
        inva2=np.tile(L2["inva"][None, :], (P, 1)).astype(np.float32),
        iota=np.tile(np.arange(MAXD, dtype=np.float32)[None, :],
                     (P, NCH)).astype(np.float32),
        ident=np.eye(P, dtype=np.float32),
        ones1=np.ones((P, 1), np.float32),
    )

    in_maps = []
    nchu_max = np.zeros((cfg.NSC, 4), np.int64)
    for c in range(cfg.n_cores):
        m = np.where(c_of == c)[0]
        packed, nchu = _pack_core(cfg, src_pid[m], dst_rel[m].astype(np.int64),
                                  ea[m])
        nchu_max = np.maximum(nchu_max, nchu)
        xo = np.zeros((cfg.NPC, F), np.float32)
        xo[:cfg.RPC] = x[c * cfg.RPC:(c + 1) * cfg.RPC]
        xoT = np.ascontiguousarray(
            np.concatenate([xo.T, np.ones((1, cfg.NPC), np.float32)], 0))
        in_maps.append(dict(xoT=xoT, **packed, **shared))
    counts = tuple(tuple(int(v) for v in row) for row in nchu_max)
    return in_maps, L1, L2, counts


# ---------------------------------------------------------------------------
# device kernel
# ---------------------------------------------------------------------------

def build_kernel(cfg, npos1, npos2, counts=None, lrelu_native=True,
                 phases="all"):
    assert 0 < npos1 < cfg.F and 0 < npos2 < cfg.F
    if counts is None:
        counts = tuple((NCH,) * 4 for _ in range(cfg.NSC))
    nc = bacc.Bacc("TRN2", target_bir_lowering=False, debug=False,
                   num_devices=cfg.n_cores, num_swdge_queues=4)
    F, ED, NPC, NSC_ = cfg.F, cfg.ED, cfg.NPC, cfg.NSC
    f16, f32, i16 = mybir.dt.float16, mybir.dt.float32, mybir.dt.int16
    NBLK = NPC // P
    npos_l = [npos1, npos2]

    ein = lambda n, s, d: nc.dram_tensor(n, s, d, kind="ExternalInput")
    t_xoT = ein("xoT", [F + 1, NPC], f32)
    t_w = {n: ein(n, [F + 1, F], f32)
           for n in ["Wlb1", "Wrb1", "R1", "Wlb2", "Wrb2", "R2"]}
    t_We = {n: ein(n, [ED, F], f16) for n in ["We1", "We2"]}
    t_inva = {n: ein(n, [P, F], f32) for n in ["inva1", "inva2"]}
    t_iota = ein("iota", [P, NCH * MAXD], f32)
    t_ident = ein("ident", [P, P], f32)
    t_ones1 = ein("ones1", [P, 1], f32)
    t_srcg = ein("srcg", [NSC_, 4, P, Q // 16], i16)
    t_dstg = ein("dstg", [NSC_, P, SC // 16], i16)
    t_scat = ein("scat", [NSC_, 4, P, MAXD * NCH // 16], i16)
    t_slot = ein("slot", [NSC_, P, SC // P], f32)
    t_eaT = ein("eaT", [ED, NSC_ * SC], f16)
    t_out = nc.dram_tensor("out", [NPC, F], f32, kind="ExternalOutput")

    xl_own = [nc.dram_tensor(f"xl_own{l}", [NPC, F], f32) for l in (0, 1)]
    xl_full = [nc.dram_tensor(f"xl_full{l}", [cfg.NPAD, F], f32,
                              addr_space="Shared") for l in (0, 1)]
    xr_own = [nc.dram_tensor(f"xr_own{l}", [NPC, F], f32) for l in (0, 1)]
    resid = [nc.dram_tensor(f"resid{l}", [NPC, F], f32) for l in (0, 1)]
    acc = [[nc.dram_tensor(f"acc{l}_{t}", [NPC, P], f32) for t in (0, 1)]
           for l in (0, 1)]
    hoT_dram = nc.dram_tensor("hoT_dram", [F + 1, NPC], f32)
    rg = [list(range(cfg.n_cores))]

    with tile.TileContext(nc) as tc:
        with (
            tc.tile_pool(name="const", bufs=1) as cpool,
            tc.tile_pool(name="io", bufs=4) as io,
            tc.tile_pool(name="big", bufs=3) as big,
            tc.tile_pool(name="mid", bufs=2) as mid,
            tc.tile_pool(name="ps", bufs=2, space="PSUM") as psp,
        ):
            nc.gpsimd.load_library(library_config.mlp)

            def stage(t, shape, dt, tag):
                s = cpool.tile(shape, dt, tag=tag, name=tag)
                nc.sync.dma_start(out=s[:], in_=t[:])
                return s

            s_w = {n: stage(t, [F + 1, F], f32, f"c_{n}")
                   for n, t in t_w.items()}
            s_We = {n: stage(t, [ED, F], f16, f"c_{n}")
                    for n, t in t_We.items()}
            s_inva = {n: stage(t, [P, F], f32, f"c_{n}")
                      for n, t in t_inva.items()}
            s_iota = stage(t_iota, [P, NCH * MAXD], f32, "c_iota")
            s_ident = stage(t_ident, [P, P], f32, "c_ident")
            s_ones1 = stage(t_ones1, [P, 1], f32, "c_ones1")

            # zero the accumulator tables
            zt = cpool.tile([P, 1024], f32, tag="c_zero")
            nc.vector.memset(zt[:], 0)
            for l in (0, 1):
                for t in (0, 1):
                    flat = acc[l][t][:].rearrange("a d -> (a d)")
                    tot, per = NPC * P, P * 1024
                    nst = (tot + per - 1) // per
                    for si in range(nst):
                        lo, hi = si * per, min((si + 1) * per, tot)
                        nc.sync.dma_start(
                            out=flat[lo:hi].rearrange("(p w) -> p w", p=P),
                            in_=zt[:, :(hi - lo) // P])

            souts = [[cpool.tile([P, 4 * P], f32, tag=f"c_so{u}_{par}",
                                 name=f"so{u}_{par}")
                      for par in (0, 1)] for u in range(4)]
            for u in range(4):
                for par in (0, 1):
                    nc.vector.memset(souts[u][par][:], 0)

            onerow = cpool.tile([P, NPC // P], f32, tag="c_onerow")
            nc.vector.memset(onerow[:], 1.0)
            nc.sync.dma_start(
                out=hoT_dram[F:F + 1, :].rearrange("a (p w) -> (a p) w", p=P),
                in_=onerow[:])

            def node_phase(l):
                src_T = t_xoT if l == 0 else hoT_dram
                # pass 1: xl only, then kick the AllGather early
                for k in range(NBLK):
                    lt = mid.tile([F + 1, P], f32, tag="lhsT")
                    nc.sync.dma_start(out=lt[:],
                                      in_=src_T[:, k * P:(k + 1) * P])
                    ps = psp.tile([P, F], f32, tag="misc")
                    nc.tensor.matmul(ps[:], lhsT=lt[:],
                                     rhs=s_w[f"Wlb{l + 1}"][:],
                                     start=True, stop=True)
                    st = mid.tile([P, F], f32, tag="xlstage")
                    nc.scalar.activation(st[:], ps[:], AF.Copy)
                    nc.sync.dma_start(out=xl_own[l][k * P:(k + 1) * P, :],
                                      in_=st[:])
                nc.gpsimd.collective_compute(
                    "AllGather", mybir.AluOpType.bypass, replica_groups=rg,
                    ins=[xl_own[l][:]], outs=[xl_full[l][:]])
                # pass 2: xr + residual, overlapped with the collective
                for k in range(NBLK):
                    lt = mid.tile([F + 1, P], f32, tag="lhsT")
                    nc.sync.dma_start(out=lt[:],
                                      in_=src_T[:, k * P:(k + 1) * P])
                    ps = psp.tile([P, 2 * F], f32, tag="misc")
                    nc.tensor.matmul(ps[:, 0:F], lhsT=lt[:],
                                     rhs=s_w[f"Wrb{l + 1}"][:],
                                     start=True, stop=True)
                    nc.tensor.matmul(ps[:, F:2 * F], lhsT=lt[:],
                                     rhs=s_w[f"R{l + 1}"][:],
                                     start=True, stop=True)
                    st = mid.tile([P, 2 * F], f32, tag="nodestage")
                    nc.vector.tensor_copy(st[:, 0:F], ps[:, 0:F])
                    nc.scalar.activation(st[:, F:2 * F], ps[:, F:2 * F],
                                         AF.Copy)
                    blk = slice(k * P, (k + 1) * P)
                    nc.sync.dma_start(out=xr_own[l][blk, :], in_=st[:, 0:F])
                    nc.sync.dma_start(out=resid[l][blk, :], in_=st[:, F:2 * F])

            def edge_phase(l):
                npos = npos_l[l]
                we = s_We[f"We{l + 1}"]
                for sc in range(NSC_):
                    cnts = counts[sc]
                    if max(cnts) == 0:
                        continue
                    slot_t = io.tile([P, SC // P], f32, tag="slot")
                    nc.sync.dma_start(out=slot_t[:], in_=t_slot[sc])
                    dstg_t = io.tile([P, SC // 16], i16, tag="dstg")
                    nc.sync.dma_start(out=dstg_t[:], in_=t_dstg[sc])

                    xl_t = big.tile([P, SC // P, F], f32, tag="xl")
                    xr_t = big.tile([P, SC // P, F], f32, tag="xr")
                    for b in range(4):
                        nq = cnts[b] * (Q // NCH)
                        if nq == 0:
                            continue
                        sg = io.tile([P, Q // 16], i16, tag="srcg")
                        nc.sync.dma_start(out=sg[:, : nq // 16],
                                          in_=t_srcg[sc, b][:, : nq // 16])
                        nc.gpsimd.dma_gather(
                            out_ap=xl_t[:, b * (Q // P):
                                        b * (Q // P) + nq // P, :],
                            in_ap=xl_full[l][b * cfg.BUCK:(b + 1) * cfg.BUCK, :],
                            idxs_ap=sg[:, : nq // 16], num_idxs=nq,
                            num_idxs_reg=nq,
                            elem_size=F, single_packet=False, queue_num=b)
                    for b in range(4):
                        nq = cnts[b] * (Q // NCH)
                        if nq == 0:
                            continue
                        nc.gpsimd.dma_gather(
                            out_ap=xr_t[:, b * (Q // P):
                                        b * (Q // P) + nq // P, :],
                            in_ap=xr_own[l][:],
                            idxs_ap=dstg_t[:, b * (Q // 16):
                                           b * (Q // 16) + nq // 16],
                            num_idxs=nq, num_idxs_reg=nq, elem_size=F,
                            single_packet=False, queue_num=b)

                    s_t = mid.tile([P, SC // P], f32, tag="s")
                    ex_t = mid.tile([P, SC // P], f32, tag="ex")
                    for u in range(4):
                        ncb = cnts[u]
                        if ncb == 0:
                            continue
                        usl = slice(u * NCH, u * NCH + ncb)
                        ea_t = io.tile([ED, Q], f16, tag="ea")
                        nc.sync.dma_start(
                            out=ea_t[:, : ncb * P],
                            in_=t_eaT[:, sc * SC + u * Q:
                                      sc * SC + u * Q + ncb * P])
                        mps = psp.tile([P, NCH, F], f32, tag="mps")
                        for jj in range(ncb):
                            col = jj * P
                            nc.tensor.matmul(mps[:, jj, :],
                                             lhsT=ea_t[:, col:col + P],
                                             rhs=we[:], start=True, stop=True)
                        m_t = mid.tile([P, NCH, F], f32, tag="m")
                        nc.vector.tensor_add(m_t[:, :ncb, :], xl_t[:, usl, :],
                                             xr_t[:, usl, :])
                        nc.vector.tensor_add(m_t[:, :ncb, :], m_t[:, :ncb, :],
                                             mps[:, :ncb, :])
                        mlr = mid.tile([P, NCH, F], f32, tag="mlr")
                        if lrelu_native:
                            nc.scalar.activation(mlr[:, :ncb, :],
                                                 m_t[:, :ncb, :], AF.Prelu,
                                                 alpha=NEG)
                        else:
                            nc.scalar.activation(mlr[:, :ncb, :],
                                                 m_t[:, :ncb, :], AF.Relu)
                        rp = mid.tile([P, NCH], f32, tag="rpos")
                        rn = mid.tile([P, NCH], f32, tag="rneg")
                        nc.vector.tensor_reduce(
                            rp[:, :ncb], mlr[:, :ncb, 0:npos],
                            axis=mybir.AxisListType.X, op=mybir.AluOpType.add)
                        nc.vector.tensor_reduce(
                            rn[:, :ncb], mlr[:, :ncb, npos:F],
                            axis=mybir.AxisListType.X, op=mybir.AluOpType.add)
                        nc.vector.tensor_sub(s_t[:, usl], rp[:, :ncb],
                                             rn[:, :ncb])
                        if not lrelu_native:
                            # lrelu(x) = NEG*x + (1-NEG)*relu(x):
                            # s = (1-NEG)*s_relu + NEG*(sum_pos m - sum_neg m)
                            rp2 = mid.tile([P, NCH], f32, tag="rpos2")
                            rn2 = mid.tile([P, NCH], f32, tag="rneg2")
                            nc.vector.tensor_reduce(
                                rp2[:], m_t[:, :, 0:npos],
                                axis=mybir.AxisListType.X,
                                op=mybir.AluOpType.add)
                            nc.vector.tensor_reduce(
                                rn2[:], m_t[:, :, npos:F],
                                axis=mybir.AxisListType.X,
                                op=mybir.AluOpType.add)
                            nc.vector.tensor_sub(rp2[:], rp2[:], rn2[:])
                            nc.vector.tensor_scalar_mul(
                                s_t[:, usl], s_t[:, usl], 1.0 - NEG)
                            nc.vector.tensor_scalar_mul(rp2[:], rp2[:], NEG)
                            nc.vector.tensor_add(s_t[:, usl], s_t[:, usl],
                                                 rp2[:])
                    nc.scalar.activation(ex_t[:], s_t[:], AF.Exp)

                    for u in range(4):
                        ncb = cnts[u]
                        if ncb == 0:
                            continue
                        usl = slice(u * NCH, u * NCH + ncb)
                        nblk4 = (ncb + 3) // 4
                        stair = mid.tile([P, NCH, MAXD], f32, tag="stair")
                        nc.vector.tensor_tensor(
                            out=stair[:, :ncb, :],
                            in0=s_iota[:].rearrange("p (c k) -> p c k",
                                                    k=MAXD)[:, :ncb, :],
                            in1=slot_t[:, usl].to_broadcast([P, ncb, MAXD]),
                            op=mybir.AluOpType.is_equal)
                        nc.vector.tensor_tensor(
                            out=stair[:, :ncb, :], in0=stair[:, :ncb, :],
                            in1=ex_t[:, usl].to_broadcast([P, ncb, MAXD]),
                            op=mybir.AluOpType.mult)
                        sps = psp.tile([P, 4 * P], f32, tag="sps")
                        for c in range(ncb):
                            pb, fb = 32 * (c % 4), P * (c // 4)
                            nc.tensor.matmul(
                                sps[pb:pb + 32, fb:fb + F],
                                lhsT=stair[:, c, :],
                                rhs=xl_t[:, u * NCH + c, :],
                                start=True, stop=True,
                                tile_position=(0, pb))
                            nc.tensor.matmul(
                                sps[pb:pb + 32, fb + F:fb + F + 1],
                                lhsT=stair[:, c, :],
                                rhs=s_ones1[:], start=True, stop=True,
                                tile_position=(0, pb))
                        so = souts[u][sc % 2]
                        nc.scalar.activation(
                            so[:].rearrange("p (c d) -> p c d",
                                            d=P)[:, :nblk4, 0:65],
                            sps[:].rearrange("p (c d) -> p c d",
                                             d=P)[:, :nblk4, 0:65],
                            AF.Copy)
                        sct = io.tile([P, MAXD * NCH // 16], i16, tag="sct")
                        nc.sync.dma_start(
                            out=sct[:, : MAXD * ncb // 16],
                            in_=t_scat[sc, u][:, : MAXD * ncb // 16])
                        nc.gpsimd.dma_scatter_add(
                            out_ap=acc[l][u // 2][:],
                            in_ap=so[:].rearrange("p (c d) -> p c d",
                                                  d=P)[:, :nblk4, :],
                            idxs_ap=sct[:, : MAXD * ncb // 16],
                            num_idxs=MAXD * ncb,
                            num_idxs_reg=MAXD * ncb, elem_size=P,
                            queue_num=u)

            def epilogue(l):
                inva = s_inva[f"inva{l + 1}"]
                for k in range(NBLK):
                    blk = slice(k * P, (k + 1) * P)
                    a0 = mid.tile([P, P], f32, tag="eacc0")
                    a1 = mid.tile([P, P], f32, tag="eacc1")
                    nc.sync.dma_start(out=a0[:], in_=acc[l][0][blk, :])
                    nc.sync.dma_start(out=a1[:], in_=acc[l][1][blk, :])
                    nc.vector.tensor_add(a0[:, 0:F + 1], a0[:, 0:F + 1],
                                         a1[:, 0:F + 1])
                    nc.vector.tensor_scalar_add(a0[:, F:F + 1],
                                                a0[:, F:F + 1], 1e-30)
                    rec = mid.tile([P, 1], f32, tag="rec")
                    nc.vector.reciprocal(rec[:], a0[:, F:F + 1])
                    gt = mid.tile([P, F], f32, tag="g")
                    nc.vector.tensor_scalar_mul(gt[:], a0[:, 0:F], rec[:])
                    nc.vector.tensor_mul(gt[:], gt[:], inva[:])
                    r_t = mid.tile([P, F], f32, tag="rt")
                    nc.sync.dma_start(out=r_t[:], in_=resid[l][blk, :])
                    nc.vector.tensor_add(gt[:], gt[:], r_t[:])
                    if l == 0:
                        h = mid.tile([P, F], f32, tag="h")
                        nc.scalar.activation(h[:], gt[:], AF.Relu)
                        tp = psp.tile([F, P], f32, tag="misc")
                        nc.tensor.transpose(out=tp[:], in_=h[:],
                                            identity=s_ident[:])
                        hs = mid.tile([F, P], f32, tag="hstage")
                        nc.scalar.activation(hs[:], tp[:], AF.Copy)
                        nc.sync.dma_start(
                            out=hoT_dram[0:F, k * P:(k + 1) * P], in_=hs[:])
                    else:
                        nc.sync.dma_start(out=t_out[blk, :], in_=gt[:])

            if phases == "node":
                node_phase(0)
                epilogue(0)
                node_phase(1)
                epilogue(1)
            elif phases == "l1":
                node_phase(0)
                edge_phase(0)
                epilogue(0)
                node_phase(1)
                epilogue(1)
            else:
                node_phase(0)
                edge_phase(0)
                epilogue(0)
                node_phase(1)
                edge_phase(1)
                epilogue(1)

    nc.compile()
    return nc


# ---------------------------------------------------------------------------
# entry point
# ---------------------------------------------------------------------------

_CACHED = {}


def run(cfg, inp, trace=False, lrelu_native=True, phases="all"):
    in_maps, L1, L2, counts = prep_inputs(cfg, inp)
    key = (cfg, L1["npos"], L2["npos"], counts, lrelu_native, phases)
    if key not in _CACHED:
        _CACHED[key] = build_kernel(cfg, L1["npos"], L2["npos"], counts,
                                    lrelu_native=lrelu_native, phases=phases)
    nc = _CACHED[key]
    res = run_bass_kernel_spmd(nc, in_maps, list(range(cfg.n_cores)),
                               trace=trace)
    out = np.concatenate(
        [res.results[c]["out"][:cfg.RPC] for c in range(cfg.n_cores)], 0)
    out = out[:cfg.N]
    unperm = np.argsort(L2["perm"])
    return np.ascontiguousarray(out[:, unperm]), res


def kernel(**inputs) -> np.ndarray:
    out, _ = run(FULL, inputs)
    return out

